# revision 5
# baseline (speedup 1.0000x reference)
"""TRN2 Bass kernel for nn_FP8LinearWrapper: y = x @ (w_fp8 * inv_scale).T + bias.

Strategy (8 NeuronCores, SPMD):
  - Data-parallel over the flattened token dim: x [4,2048,4096] -> [8192,4096],
    1024 rows per core. Weights/bias replicated to every core.
  - Per core: SINGLE-pass bf16 matmul. x is rounded to bf16 on device (ACT
    cast) and transposed by the DMA XBAR (dma_start_transpose) into a
    resident SBUF operand; the fp8 weight is fed directly as the matmul
    moving operand (mixed bf16 x fp8e4 matmul, verified bit-exact on HW);
    accumulation in fp32 PSUM. Error is dominated by the bf16 rounding of x:
    ~1.7e-3 rel absmax vs the 2e-2 gate (the dual-pass hi+lo scheme this
    replaces was 2x the PE work for accuracy the gate does not need).
  - The fp8 weight bytes are jax float8_e4m3fn (max 448). TRN2's fp8e4 decode
    is IEEE e4m3 (max 240), so the host re-encodes each byte via a LUT to the
    e4m3 bits of (value/2) - exact for all normals - and the kernel folds the
    missing *2 into the output scale. w is passed pre-transposed/pre-blocked
    (weight layout prep, as for any serving stack).
  - The PE runs ONLY the 2048 [128k x 128m x 512o] matmuls (216 ns each at
    the bf16 streaming roofline, ~443 us/core): transposes ride the DMA
    XBAR (14 ns per 16x128 tile, overlapped), casts ride the idle ACT
    engine, evictions ride the DVE.
  - xt layout [P, MT, 4, 8, P] keeps every XBAR-transpose destination
    per-partition contiguous (non-contiguous xbar dst is a known HW-wrong
    pattern), while matmul stationary slices xt[:, mt, c, kk, :] stay 2D
    contiguous.
  - m-tile PAIRS share one 2-bank PSUM tile (2 x 32 accumulating matmuls)
    with a single fused (psum * 2*inv_scale) + bias DVE eviction covering
    both banks, halving group-boundary syncs.
"""

import os
import sys

for _p in (
    "/opt/trn_rl_repo",
    "/root/.axon_site",
    "/root/.axon_site/_ro/trn_rl_repo",
    "/root/.axon_site/_ro/pypackages",
):
    if os.path.isdir(_p) and _p not in sys.path:
        sys.path.append(_p)

import numpy as np
import ml_dtypes

B, S, DI, DO = 4, 2048, 4096, 4096
NCORES = 8
M = B * S            # 8192
MC = M // NCORES     # 1024 rows per core
P = 128
KT = DI // P         # 32 k-tiles
MT = MC // P         # 8 m-tiles per core
OBW = 512            # o-block width
OB = DO // OBW       # 8 o-blocks
WCK = 4              # k-tiles per weight chunk
WCH = KT // WCK      # 8 weight chunks per o-block
XC = 4               # 1024-col x chunks per m-tile
XK = KT // XC        # 8 k-tiles per x chunk

_STATE = {}


def _build_program():
    import concourse.bass as bass
    import concourse.mybir as mybir
    import concourse.tile as tile
    from concourse import bacc

    dt = mybir.dt
    F32, BF16, FP8 = dt.float32, dt.bfloat16, dt.float8e4

    nc = bacc.Bacc(target_bir_lowering=False)

    x_in = nc.dram_tensor("x", [MC, DI], F32, kind="ExternalInput")
    w_in = nc.dram_tensor("w", [OB, P, KT, OBW], FP8, kind="ExternalInput")
    s_in = nc.dram_tensor("s", [P, 1], F32, kind="ExternalInput")
    b_in = nc.dram_tensor("b", [P, DO], F32, kind="ExternalInput")
    y_out = nc.dram_tensor("y", [MC, DO], F32, kind="ExternalOutput")

    with tile.TileContext(nc) as tc:
        with (
            tc.tile_pool(name="const", bufs=1) as const,
            tc.tile_pool(name="xt_pool", bufs=1) as xt_pool,
            tc.tile_pool(name="xin_pool", bufs=3) as xin_pool,
            tc.tile_pool(name="xbf_pool", bufs=3) as xbf_pool,
            tc.tile_pool(name="w8_pool", bufs=18) as w8_pool,
            tc.tile_pool(name="bias_pool", bufs=2) as bias_pool,
            tc.tile_pool(name="out_pool", bufs=2) as out_pool,
            tc.tile_pool(name="mm_ps_pool", bufs=3, space="PSUM") as mm_ps_pool,
        ):
            s_t = const.tile([P, 1], F32)
            nc.sync.dma_start(out=s_t, in_=s_in[:, :])
            s2 = const.tile([P, 1], F32)
            nc.scalar.mul(s2, s_t, 2.0)  # fold back the /2 from the fp8 re-encode

            # resident transposed bf16 x: [d-part, mt, chunk, kt-in-chunk, m]
            xt = xt_pool.tile([P, MT, XC, XK, P], BF16)

            def load_wchunks(ob):
                bias2 = bias_pool.tile([P, 2 * OBW], F32, name=f"bias2_{ob}", tag="bias")
                for h in range(2):
                    nc.sync.dma_start(
                        out=bias2[:, h * OBW:(h + 1) * OBW],
                        in_=b_in[:, ob * OBW:(ob + 1) * OBW],
                    )
                wchunks = []
                for c in range(WCH):
                    w8c = w8_pool.tile([P, WCK, OBW], FP8, name=f"w8_{ob}_{c}", tag="w8")
                    nc.sync.dma_start(out=w8c, in_=w_in[ob, :, c * WCK:(c + 1) * WCK, :])
                    wchunks.append(w8c)
                return bias2, wchunks

            def x_chain(mt, first_xins=None):
                # stream one m-tile of x: DMA f32 chunk -> ACT bf16 cast ->
                # XBAR transpose into the resident xt (dst per-partition
                # contiguous [XK*P] runs)
                for c in range(XC):
                    if first_xins is not None:
                        xin = first_xins[c]
                    else:
                        xin = xin_pool.tile([P, 1024], F32, name=f"xin_{mt}_{c}", tag="xin")
                        nc.sync.dma_start(
                            out=xin,
                            in_=x_in[mt * P:(mt + 1) * P, c * 1024:(c + 1) * 1024],
                        )
                    xbf = xbf_pool.tile([P, 1024], BF16, name=f"xbf_{mt}_{c}", tag="xbf")
                    nc.scalar.copy(xbf, xin)
                    # scalar (Activation) HWDGE ring: keeps the xbar's small
                    # packets off the sync ring that carries the bulk loads
                    nc.scalar.dma_start_transpose(xt[:, mt, c, :, :], xbf)

            def mm_pair(ob, mt0, bias2, wchunks):
                # two m-tile groups share one 2-bank PSUM tile and a single
                # fused eviction -> half the group-boundary syncs on PE
                ps = mm_ps_pool.tile([P, 2 * OBW], F32, name=f"ps_{ob}_{mt0}", tag="ps")
                for h, mt in ((0, mt0), (1, mt0 + 1)):
                    ps_h = ps[:, h * OBW:(h + 1) * OBW]
                    for kt in range(KT):
                        wb_sl = wchunks[kt // WCK][:, kt % WCK, :]
                        nc.tensor.matmul(
                            ps_h, xt[:, mt, kt // XK, kt % XK, :], wb_sl,
                            start=(kt == 0), stop=(kt == KT - 1),
                            skip_group_check=True,
                        )
                out_sb = out_pool.tile([P, 2 * OBW], F32, name=f"o_{ob}_{mt0}", tag="out")
                nc.vector.scalar_tensor_tensor(
                    out_sb, ps, s2[:, :], bias2,
                    mybir.AluOpType.mult, mybir.AluOpType.add,
                )
                for h, mt in ((0, mt0), (1, mt0 + 1)):
                    nc.sync.dma_start(
                        out=y_out[mt * P:(mt + 1) * P, ob * OBW:(ob + 1) * OBW],
                        in_=out_sb[:, h * OBW:(h + 1) * OBW],
                    )

            # ---- Phase T: stream x through cast+XBAR-transpose while the PE
            # runs o-blocks 0..1 on already-transposed m-tile pairs ----
            first_xins = []
            for c in range(XC):  # mt0's loads beat the 4MB o-block-0/1 w prefetch
                xin = xin_pool.tile([P, 1024], F32, name=f"xin_0_{c}", tag="xin")
                nc.sync.dma_start(out=xin, in_=x_in[0:P, c * 1024:(c + 1) * 1024])
                first_xins.append(xin)
            bias_w = [load_wchunks(0), load_wchunks(1)]
            for mt0 in range(0, MT, 2):
                x_chain(mt0, first_xins if mt0 == 0 else None)
                x_chain(mt0 + 1)
                for ob in (0, 1):
                    mm_pair(ob, mt0, *bias_w[ob])

            # ---- Phase B: o-blocks 2..7 stream w fp8 from DRAM against the
            # resident xt ----
            for ob in range(2, OB):
                bias2, wchunks = load_wchunks(ob)
                for mt0 in range(0, MT, 2):
                    mm_pair(ob, mt0, bias2, wchunks)

    nc.finalize()
    return nc


def _get_program():
    if "nc" not in _STATE:
        _STATE["nc"] = _build_program()
    return _STATE["nc"]


def _prep_weights(weight_fp8):
    """Re-encode jax e4m3fn bytes as IEEE-e4m3 bytes of value/2 (exact for
    normals), transpose to [d, o], and block to [ob, p, kt, obw] so each
    o-block DMA reads 2KB-contiguous per-partition lines."""
    bits = np.arange(256, dtype=np.uint8)
    vals = bits.view(ml_dtypes.float8_e4m3fn).astype(np.float32) * 0.5
    lut = vals.astype(ml_dtypes.float8_e4m3).view(np.uint8)

    wb = np.asarray(weight_fp8).view(np.uint8)          # [DO, DI]
    w2t = np.ascontiguousarray(lut[wb].T)               # [DI, DO]
    w_pre = np.ascontiguousarray(
        w2t.reshape(KT, P, OB, OBW).transpose(2, 1, 0, 3)
    )                                                   # [OB, P, KT, OBW]
    return w_pre.view(ml_dtypes.float8_e4m3)


def kernel(x, weight_fp8, weight_inv_scale, bias):
    from concourse.bass_utils import run_bass_kernel_spmd

    try:
        import jax
        jax.config.update("jax_compilation_cache_dir", "/tmp/jax_neff_cache")
        jax.config.update("jax_persistent_cache_min_entry_size_bytes", 0)
        jax.config.update("jax_persistent_cache_min_compile_time_secs", 0.0)
    except Exception:
        pass

    nc = _get_program()

    x_np = np.asarray(x, dtype=np.float32).reshape(M, DI)
    w_pre = _prep_weights(weight_fp8)
    s_b = np.ascontiguousarray(
        np.broadcast_to(
            np.asarray(weight_inv_scale, dtype=np.float32).reshape(1, 1), (P, 1)
        )
    )
    b_b = np.ascontiguousarray(
        np.broadcast_to(np.asarray(bias, dtype=np.float32), (P, DO))
    )

    core_ids = list(range(NCORES))
    in_maps = [
        {"x": x_np[c * MC:(c + 1) * MC], "w": w_pre, "s": s_b, "b": b_b}
        for c in core_ids
    ]

    last_err = None
    for _attempt in range(3):
        try:
            res = run_bass_kernel_spmd(nc, in_maps, core_ids)
            break
        except Exception as e:  # device wedge (NRT_EXEC_UNIT_UNRECOVERABLE): reset + retry
            last_err = e
            try:
                import jax
                import time
                jax.clear_backends()
                time.sleep(3.0)
            except Exception:
                pass
    else:
        raise last_err

    y = np.concatenate([res.results[c]["y"] for c in core_ids], axis=0)
    return y.reshape(B, S, DO)


# revision 6
# speedup vs baseline: 1.2371x; 1.2371x over previous
"""TRN2 Bass kernel for nn_FP8LinearWrapper: y = x @ (w_fp8 * inv_scale).T + bias.

Strategy (8 NeuronCores, SPMD):
  - Data-parallel over the flattened token dim: x [4,2048,4096] -> [8192,4096],
    1024 rows per core. Weights/bias replicated to every core.
  - Per core: SINGLE-pass bf16 matmul. x is rounded to bf16 on device (DVE
    cast), PE-transposed in bf16 (53 ns per 128x128 tile), and kept resident
    in SBUF; the fp8 weight is fed directly as the matmul moving operand
    (mixed bf16 x fp8e4 matmul, verified bit-exact on HW); accumulation in
    fp32 PSUM. Error is dominated by the bf16 rounding of x: ~1.7e-3 rel
    absmax vs the 2e-2 gate (the dual-pass hi+lo scheme this replaces was 2x
    the PE work for accuracy the gate does not need).
  - The fp8 weight bytes are jax float8_e4m3fn (max 448). TRN2's fp8e4 decode
    is IEEE e4m3 (max 240), so the host re-encodes each byte via a LUT to the
    e4m3 bits of (value/2) - exact for all normals - and the kernel folds the
    missing *2 into the output scale. w is passed pre-transposed/pre-blocked
    (weight layout prep, as for any serving stack).

Timing structure (~457 us/core of PE work: 2048 matmuls at the 216 ns/MM
N=512 bf16 streaming roofline + 256 bf16 transposes at 53 ns):
  - Phase T: m-tile pairs; per pair the PE runs o-blocks 0..1 (128 matmuls)
    while the NEXT pair's x streams in (DMA f32 -> DVE bf16 cast -> PE
    transpose -> ACT psum->sbuf copy). x-chains are emitted one pair BEHIND
    the matmuls so the in-order PE queue never waits on in-flight DMA, and
    the ACT copy stream (~22 us/pair) hides under the matmul window (~28 us).
  - Phase B: o-blocks 2..7 stream w fp8 from DRAM against the resident xt.
  - m-tile pairs share one 2-bank PSUM tile (2 x 32 accumulating matmuls)
    with a single fused (psum * 2*inv_scale) + bias DVE eviction covering
    both banks, halving group-boundary syncs.
  - DMA transposes (XBAR) were tried and rejected: their 256B packets cap
    aggregate DMA at ~150 GB/s (per-engine packet-rate limit), starving the
    PE in phase T.
"""

import os
import sys

for _p in (
    "/opt/trn_rl_repo",
    "/root/.axon_site",
    "/root/.axon_site/_ro/trn_rl_repo",
    "/root/.axon_site/_ro/pypackages",
):
    if os.path.isdir(_p) and _p not in sys.path:
        sys.path.append(_p)

import numpy as np
import ml_dtypes

B, S, DI, DO = 4, 2048, 4096, 4096
NCORES = 8
M = B * S            # 8192
MC = M // NCORES     # 1024 rows per core
P = 128
KT = DI // P         # 32 k-tiles
MT = MC // P         # 8 m-tiles per core
OBW = 512            # o-block width
OB = DO // OBW       # 8 o-blocks
WCK = 4              # k-tiles per weight chunk
WCH = KT // WCK      # 8 weight chunks per o-block
XC = 4               # 1024-col x chunks per m-tile
XK = KT // XC        # 8 k-tiles per x chunk

_STATE = {}


def _build_program():
    import concourse.bass as bass
    import concourse.mybir as mybir
    import concourse.tile as tile
    from concourse import bacc
    from concourse.masks import make_identity

    dt = mybir.dt
    F32, BF16, FP8 = dt.float32, dt.bfloat16, dt.float8e4

    nc = bacc.Bacc(target_bir_lowering=False)

    x_in = nc.dram_tensor("x", [MC, DI], F32, kind="ExternalInput")
    w_in = nc.dram_tensor("w", [OB, P, KT, OBW], FP8, kind="ExternalInput")
    s_in = nc.dram_tensor("s", [P, 1], F32, kind="ExternalInput")
    b_in = nc.dram_tensor("b", [P, DO], F32, kind="ExternalInput")
    y_out = nc.dram_tensor("y", [MC, DO], F32, kind="ExternalOutput")

    with tile.TileContext(nc) as tc:
        with (
            tc.tile_pool(name="const", bufs=1) as const,
            tc.tile_pool(name="xt_pool", bufs=1) as xt_pool,
            tc.tile_pool(name="xin_pool", bufs=3) as xin_pool,
            tc.tile_pool(name="xcb_pool", bufs=3) as xcb_pool,
            tc.tile_pool(name="w8_pool", bufs=18) as w8_pool,
            tc.tile_pool(name="bias_pool", bufs=2) as bias_pool,
            tc.tile_pool(name="out_pool", bufs=2) as out_pool,
            tc.tile_pool(name="tp_ps_pool", bufs=4, space="PSUM") as tp_ps_pool,
            tc.tile_pool(name="mm_ps_pool", bufs=2, space="PSUM") as mm_ps_pool,
        ):
            ident = const.tile([P, P], BF16)
            make_identity(nc, ident)
            s_t = const.tile([P, 1], F32)
            nc.sync.dma_start(out=s_t, in_=s_in[:, :])
            s2 = const.tile([P, 1], F32)
            nc.scalar.mul(s2, s_t, 2.0)  # fold back the /2 from the fp8 re-encode

            # resident transposed bf16 x: [d-part, mt, chunk, kt-in-chunk, m]
            xt = xt_pool.tile([P, MT, XC, XK, P], BF16)

            def load_wchunks(ob):
                bias2 = bias_pool.tile([P, 2 * OBW], F32, name=f"bias2_{ob}", tag="bias")
                for h in range(2):
                    nc.sync.dma_start(
                        out=bias2[:, h * OBW:(h + 1) * OBW],
                        in_=b_in[:, ob * OBW:(ob + 1) * OBW],
                    )
                wchunks = []
                for c in range(WCH):
                    w8c = w8_pool.tile([P, WCK, OBW], FP8, name=f"w8_{ob}_{c}", tag="w8")
                    nc.sync.dma_start(out=w8c, in_=w_in[ob, :, c * WCK:(c + 1) * WCK, :])
                    wchunks.append(w8c)
                return bias2, wchunks

            def x_chain(mt, first_xins=None):
                # stream one m-tile of x: DMA f32 chunk -> DVE bf16 cast ->
                # PE transpose (bf16) -> ACT psum->sbuf copy into resident xt
                for c in range(XC):
                    if first_xins is not None:
                        xin = first_xins[c]
                    else:
                        xin = xin_pool.tile([P, 1024], F32, name=f"xin_{mt}_{c}", tag="xin")
                        nc.sync.dma_start(
                            out=xin,
                            in_=x_in[mt * P:(mt + 1) * P, c * 1024:(c + 1) * 1024],
                        )
                    xcb = xcb_pool.tile([P, 1024], BF16, name=f"xcb_{mt}_{c}", tag="xcb")
                    nc.vector.tensor_copy(out=xcb, in_=xin)
                    for kk in range(XK):
                        tp = tp_ps_pool.tile([P, P], BF16, name=f"tp_{mt}_{c}_{kk}", tag="tp")
                        nc.tensor.matmul(
                            tp, xcb[:, kk * P:(kk + 1) * P], ident,
                            is_transpose=True, skip_group_check=True,
                        )
                        nc.scalar.copy(xt[:, mt, c, kk, :], tp)

            def mm_pair(ob, mt0, bias2, wchunks):
                # two m-tile groups share one 2-bank PSUM tile and a single
                # fused eviction -> half the group-boundary syncs on PE
                ps = mm_ps_pool.tile([P, 2 * OBW], F32, name=f"ps_{ob}_{mt0}", tag="ps")
                for h, mt in ((0, mt0), (1, mt0 + 1)):
                    ps_h = ps[:, h * OBW:(h + 1) * OBW]
                    for kt in range(KT):
                        wb_sl = wchunks[kt // WCK][:, kt % WCK, :]
                        nc.tensor.matmul(
                            ps_h, xt[:, mt, kt // XK, kt % XK, :], wb_sl,
                            start=(kt == 0), stop=(kt == KT - 1),
                            skip_group_check=True,
                        )
                out_sb = out_pool.tile([P, 2 * OBW], F32, name=f"o_{ob}_{mt0}", tag="out")
                nc.vector.scalar_tensor_tensor(
                    out_sb, ps, s2[:, :], bias2,
                    mybir.AluOpType.mult, mybir.AluOpType.add,
                )
                for h, mt in ((0, mt0), (1, mt0 + 1)):
                    nc.sync.dma_start(
                        out=y_out[mt * P:(mt + 1) * P, ob * OBW:(ob + 1) * OBW],
                        in_=out_sb[:, h * OBW:(h + 1) * OBW],
                    )

            # ---- Phase T: software-pipelined; pair p's matmuls (o-blocks
            # 0..1) run while pair p+1's x streams through cast/transpose ----
            first_xins = []
            for c in range(XC):  # mt0's loads beat the 4MB o-block-0/1 w prefetch
                xin = xin_pool.tile([P, 1024], F32, name=f"xin_0_{c}", tag="xin")
                nc.sync.dma_start(out=xin, in_=x_in[0:P, c * 1024:(c + 1) * 1024])
                first_xins.append(xin)
            bias_w = [load_wchunks(0), load_wchunks(1)]
            x_chain(0, first_xins)
            x_chain(1)
            for mt0 in range(0, MT, 2):
                for ob in (0, 1):
                    mm_pair(ob, mt0, *bias_w[ob])
                if mt0 + 2 < MT:
                    x_chain(mt0 + 2)
                    x_chain(mt0 + 3)

            # ---- Phase B: o-blocks 2..7 stream w fp8 from DRAM against the
            # resident xt ----
            for ob in range(2, OB):
                bias2, wchunks = load_wchunks(ob)
                for mt0 in range(0, MT, 2):
                    mm_pair(ob, mt0, bias2, wchunks)

    nc.finalize()
    return nc


def _get_program():
    if "nc" not in _STATE:
        _STATE["nc"] = _build_program()
    return _STATE["nc"]


def _prep_weights(weight_fp8):
    """Re-encode jax e4m3fn bytes as IEEE-e4m3 bytes of value/2 (exact for
    normals), transpose to [d, o], and block to [ob, p, kt, obw] so each
    o-block DMA reads 2KB-contiguous per-partition lines."""
    bits = np.arange(256, dtype=np.uint8)
    vals = bits.view(ml_dtypes.float8_e4m3fn).astype(np.float32) * 0.5
    lut = vals.astype(ml_dtypes.float8_e4m3).view(np.uint8)

    wb = np.asarray(weight_fp8).view(np.uint8)          # [DO, DI]
    w2t = np.ascontiguousarray(lut[wb].T)               # [DI, DO]
    w_pre = np.ascontiguousarray(
        w2t.reshape(KT, P, OB, OBW).transpose(2, 1, 0, 3)
    )                                                   # [OB, P, KT, OBW]
    return w_pre.view(ml_dtypes.float8_e4m3)


def kernel(x, weight_fp8, weight_inv_scale, bias):
    from concourse.bass_utils import run_bass_kernel_spmd

    try:
        import jax
        jax.config.update("jax_compilation_cache_dir", "/tmp/jax_neff_cache")
        jax.config.update("jax_persistent_cache_min_entry_size_bytes", 0)
        jax.config.update("jax_persistent_cache_min_compile_time_secs", 0.0)
    except Exception:
        pass

    nc = _get_program()

    x_np = np.asarray(x, dtype=np.float32).reshape(M, DI)
    w_pre = _prep_weights(weight_fp8)
    s_b = np.ascontiguousarray(
        np.broadcast_to(
            np.asarray(weight_inv_scale, dtype=np.float32).reshape(1, 1), (P, 1)
        )
    )
    b_b = np.ascontiguousarray(
        np.broadcast_to(np.asarray(bias, dtype=np.float32), (P, DO))
    )

    core_ids = list(range(NCORES))
    in_maps = [
        {"x": x_np[c * MC:(c + 1) * MC], "w": w_pre, "s": s_b, "b": b_b}
        for c in core_ids
    ]

    last_err = None
    for _attempt in range(3):
        try:
            res = run_bass_kernel_spmd(nc, in_maps, core_ids)
            break
        except Exception as e:  # device wedge (NRT_EXEC_UNIT_UNRECOVERABLE): reset + retry
            last_err = e
            try:
                import jax
                import time
                jax.clear_backends()
                time.sleep(3.0)
            except Exception:
                pass
    else:
        raise last_err

    y = np.concatenate([res.results[c]["y"] for c in core_ids], axis=0)
    return y.reshape(B, S, DO)
